# revision 3
# baseline (speedup 1.0000x reference)
"""Trainium2 Bass kernel for an image-captioning decoder:
embedding lookup -> 21-step LSTM (B=64, H=512) -> vocab projection
[1280,512]@[512,32000] -> softmax over V=32000.

Sharding: vocab-parallel across 8 cores (4000 vocab cols each); the LSTM is
computed redundantly on every core (its cost is batch-independent weight
streaming).  Softmax denominators are combined with a single tiny AllReduce.
"""

import numpy as np

import concourse.bass as bass
import concourse.mybir as mybir
import concourse.tile as tile
from concourse import bacc
from concourse.bass_utils import run_bass_kernel_spmd

# problem shapes (hardcoded per contract)
E = 512
H = 512
V = 32000
B = 64
T = 20
S = T + 1          # lstm steps (features + T embeddings)
G4 = 4 * H         # 2048 gate pre-activations
NCORES = 8
VS = V // NCORES   # 4000 vocab cols per core
RX = B * S         # 1344 rows of lstm input
RXP = 11 * 128     # padded to 1408 so row-chunks are uniform
NM_X = 11          # x row-chunks of 128 (2 steps each)
NM_L = 10          # logits row-chunks of 128 (2 timesteps each)
NCH = 8            # vocab chunks per core (4000 = 8 * 500)
CH = VS // NCH     # 500

F32 = mybir.dt.float32
F32R = mybir.dt.float32r
BF16 = mybir.dt.bfloat16
I32 = mybir.dt.int32
AF = mybir.ActivationFunctionType
ALU = mybir.AluOpType
AX = mybir.AxisListType


def _r(ap):
    """bitcast an AP to float32r for fast (1 cyc/row) PE matmuls"""
    return ap.bitcast(F32R)


def build_nc(n_cores=NCORES):
    nc = bacc.Bacc(
        "TRN2",
        target_bir_lowering=False,
        debug=False,
        enable_asserts=True,
        num_devices=n_cores,
    )

    # ---- I/O ----
    feat = nc.dram_tensor("features", [B, E], F32, kind="ExternalInput").ap()
    emb = nc.dram_tensor("emb_table", [V, E], F32, kind="ExternalInput").ap()
    idxd = nc.dram_tensor("idx_all", [128, NM_X], I32, kind="ExternalInput").ap()
    wihd = nc.dram_tensor("w_ihT", [4, 128, G4], F32R, kind="ExternalInput").ap()
    whhd = nc.dram_tensor("w_hhT", [4, 128, G4], F32R, kind="ExternalInput").ap()
    biasd = nc.dram_tensor("bias", [128, G4], F32, kind="ExternalInput").ap()
    fcwd = nc.dram_tensor("fc_wT", [4, 128, VS], F32R, kind="ExternalInput").ap()
    fcbd = nc.dram_tensor("fc_b", [1, VS], F32R, kind="ExternalInput").ap()
    idend = nc.dram_tensor("iden", [128, 128], F32, kind="ExternalInput").ap()
    idenrd = nc.dram_tensor("iden_r", [128, 128], F32R, kind="ExternalInput").ap()
    onesd = nc.dram_tensor("ones", [1, 128], F32R, kind="ExternalInput").ap()
    outd = nc.dram_tensor("out", [B, T, VS], F32, kind="ExternalOutput").ap()
    out_tbv = outd.rearrange("b t v -> t b v")  # [20, 64, VS]

    with tile.TileContext(nc) as tc:
        with (
            tc.tile_pool(name="consts", bufs=1) as constP,
            tc.tile_pool(name="state", bufs=1) as stateP,
        ):
            # ---- constants / persistent state ----
            id_sb = constP.tile([128, 128], F32)
            nc.sync.dma_start(id_sb[:], idend)
            idr_sb = constP.tile([128, 128], F32R)
            nc.sync.dma_start(idr_sb[:], idenrd)
            ones_sb = constP.tile([1, 128], F32R)
            nc.sync.dma_start(ones_sb[:], onesd)
            idx_sb = constP.tile([128, NM_X], I32)
            nc.sync.dma_start(idx_sb[:], idxd)

            hsT = stateP.tile([128, 4, T * B], F32R)  # h_1..h_20 transposed
            hT0 = stateP.tile([128, 4, B], F32R)      # h_0 transposed
            c_sb = stateP.tile([B, H], F32)          # cell state
            sums = stateP.tile([128, NM_L], F32)     # exp row-sums (local shard)
            inv = stateP.tile([128, NM_L], F32)      # 1/global row-sums

            with (
                tc.tile_pool(name="lstm", bufs=2) as lp,
                tc.tile_pool(name="xgwin", bufs=4) as xgP,
                tc.tile_pool(name="weights", bufs=1) as wP,
                tc.tile_pool(name="psA", bufs=1, space="PSUM") as psA,
                tc.tile_pool(name="psB", bufs=2, space="PSUM") as psB,
                tc.tile_pool(name="psG", bufs=1, space="PSUM") as psG,
                tc.tile_pool(name="psT", bufs=1, space="PSUM") as psT,
            ):
                bias_sb = wP.tile([128, G4], F32)
                nc.sync.dma_start(bias_sb[:], biasd)
                wih = wP.tile([128, 4, G4], F32R)
                whh = wP.tile([128, 4, G4], F32R)
                for k in range(4):
                    nc.sync.dma_start(wih[:, k, :], wihd[k])
                    nc.sync.dma_start(whh[:, k, :], whhd[k])

                # ---- phase A: gather + transpose + xgates (windowed) ----
                xg_tiles = []
                for m in range(NM_X):
                    x_raw = lp.tile([128, E], F32, tag="xraw")
                    nc.gpsimd.indirect_dma_start(
                        out=x_raw[:],
                        out_offset=None,
                        in_=emb,
                        in_offset=bass.IndirectOffsetOnAxis(
                            ap=idx_sb[:, m : m + 1], axis=0
                        ),
                    )
                    if m == 0:
                        # rows 0:64 are t=0 -> image features
                        nc.sync.dma_start(x_raw[0:B, :], feat)
                    # transpose 4 k-chunks: [128,E] -> XT [128,4,128]
                    xt_ps = psA.tile([128, 512], F32, tag="xtps")
                    for k in range(4):
                        nc.tensor.transpose(
                            xt_ps[:, k * 128 : (k + 1) * 128],
                            x_raw[:, k * 128 : (k + 1) * 128],
                            id_sb[:],
                        )
                    xt = lp.tile([128, 4, 128], F32R, tag="xt")
                    nc.vector.tensor_copy(
                        xt[:], xt_ps[:].rearrange("p (k c) -> p k c", k=4)
                    )
                    # xgates m-tile: [128, G4] = XT_m.T @ w_ihT  (+bias)
                    xg = xgP.tile([128, G4], F32R, tag="xg")
                    for n in range(4):
                        ps = psB.tile([128, 512], F32, tag="xgps")
                        for k in range(4):
                            nc.tensor.matmul(
                                ps[:],
                                xt[:, k, :],
                                wih[:, k, n * 512 : (n + 1) * 512],
                                start=(k == 0),
                                stop=(k == 3),
                            )
                        nc.vector.tensor_tensor(
                            out=xg[:, n * 512 : (n + 1) * 512],
                            in0=ps[:],
                            in1=bias_sb[:, n * 512 : (n + 1) * 512],
                            op=ALU.add,
                        )
                    xg_tiles.append(xg)

                # ---- phase B: recurrence ----
                for s in range(S):
                    if s == 0:
                        gates_ifo = xg_tiles[0][0:B, 0 : 3 * H].bitcast(F32)
                        gates_g = xg_tiles[0][0:B, 3 * H : G4].bitcast(F32)
                    else:
                        psg = psG.tile([B, G4], F32, tag="psg")
                        xg = xg_tiles[s // 2]
                        half = idr_sb[:, 64 * (s % 2) : 64 * (s % 2) + 64]
                        hprev = hT0 if s == 1 else None
                        for n in range(4):
                            sl = slice(n * 512, (n + 1) * 512)
                            nc.tensor.matmul(
                                psg[:, sl], half, xg[:, sl],
                                start=True, stop=False,
                            )
                            for k in range(4):
                                lhsT = (
                                    hprev[:, k, :]
                                    if hprev is not None
                                    else hsT[:, k, (s - 2) * B : (s - 1) * B]
                                )
                                nc.tensor.matmul(
                                    psg[:, sl], lhsT, whh[:, k, sl],
                                    start=False, stop=(k == 3),
                                )
                        gates_ifo = psg[:, 0 : 3 * H]
                        gates_g = psg[:, 3 * H : G4]

                    sig = lp.tile([B, 3 * H], F32, tag="sig")
                    nc.scalar.activation(sig[:], gates_ifo, AF.Sigmoid)
                    g = lp.tile([B, H], F32, tag="g")
                    nc.scalar.activation(g[:], gates_g, AF.Tanh)
                    t1 = lp.tile([B, H], F32, tag="t1")
                    t2 = lp.tile([B, H], F32, tag="t2")
                    if s == 0:
                        # c = i * g
                        nc.vector.tensor_tensor(
                            out=c_sb[:], in0=sig[:, 0:H], in1=g[:], op=ALU.mult
                        )
                    else:
                        nc.vector.tensor_tensor(
                            out=t1[:], in0=sig[:, H : 2 * H], in1=c_sb[:], op=ALU.mult
                        )
                        nc.vector.tensor_tensor(
                            out=t2[:], in0=sig[:, 0:H], in1=g[:], op=ALU.mult
                        )
                        nc.vector.tensor_tensor(
                            out=c_sb[:], in0=t1[:], in1=t2[:], op=ALU.add
                        )
                    tc_t = lp.tile([B, H], F32, tag="tc")
                    nc.scalar.activation(tc_t[:], c_sb[:], AF.Tanh)
                    h_sb = lp.tile([B, H], F32, tag="h")
                    nc.vector.tensor_tensor(
                        out=h_sb[:], in0=sig[:, 2 * H : 3 * H], in1=tc_t[:], op=ALU.mult
                    )
                    # transpose h -> [128, 4, 64]
                    pst = psT.tile([128, 4 * B], F32, tag="pst")
                    for k in range(4):
                        nc.tensor.transpose(
                            pst[:, k * B : (k + 1) * B],
                            h_sb[:, k * 128 : (k + 1) * 128],
                            id_sb[0:B, 0:B],
                        )
                    dst = hT0[:] if s == 0 else hsT[:, :, (s - 1) * B : s * B]
                    nc.vector.tensor_copy(
                        dst, pst[:].rearrange("p (k b) -> p k b", k=4)
                    )

            # ---- phase C: logits + exp (vocab shard) ----
            with (
                tc.tile_pool(name="fcw", bufs=1) as fwP,
                tc.tile_pool(name="smP", bufs=1) as smP,
                tc.tile_pool(name="accP", bufs=2) as accP,
                tc.tile_pool(name="psC", bufs=8, space="PSUM") as psC,
            ):
                fcb_sb = fwP.tile([1, VS], F32R)
                nc.sync.dma_start(fcb_sb[:], fcbd)
                fcw = fwP.tile([128, 4, VS], F32R)
                for k in range(4):
                    nc.sync.dma_start(fcw[:, k, :], fcwd[k])
                sm = smP.tile([128, NM_L, VS], BF16)

                for m in range(NM_L):
                    acc = accP.tile([128, NCH], F32, tag="acc")
                    for j in range(NCH):
                        sl = slice(j * CH, (j + 1) * CH)
                        ps = psC.tile([128, CH], F32, tag="lg")
                        # fc_b via K=1 matmul (resets psum), then 4 k-tiles
                        nc.tensor.matmul(
                            ps[:], ones_sb[0:1, :], fcb_sb[0:1, sl],
                            start=True, stop=False,
                        )
                        for k in range(4):
                            nc.tensor.matmul(
                                ps[:],
                                hsT[:, k, m * 128 : (m + 1) * 128],
                                fcw[:, k, sl],
                                start=False, stop=(k == 3),
                            )
                        nc.scalar.activation(
                            sm[:, m, sl], ps[:], AF.Exp,
                            accum_out=acc[:, j : j + 1],
                        )
                    nc.vector.reduce_sum(sums[:, m : m + 1], acc[:], axis=AX.X)

            # ---- phase D: allreduce sums, normalize, write out ----
            with (
                tc.tile_pool(name="dram", bufs=1, space="DRAM") as dramP,
                tc.tile_pool(name="outP", bufs=2) as outP,
                tc.tile_pool(name="smP2", bufs=1) as smP2,
            ):
                snd = dramP.tile([128, NM_L], F32)
                rcv = dramP.tile([128, NM_L], F32)
                nc.sync.dma_start(snd[:], sums[:])
                nc.gpsimd.collective_compute(
                    "AllReduce",
                    ALU.add,
                    replica_groups=[list(range(n_cores))],
                    ins=[snd.opt()],
                    outs=[rcv.opt()],
                )
                gsums = smP2.tile([128, NM_L], F32)
                nc.sync.dma_start(gsums[:], rcv[:])
                nc.vector.reciprocal(inv[:], gsums[:])

                for m in range(NM_L):
                    outf = outP.tile([128, VS], F32, tag="outf")
                    nc.vector.tensor_scalar_mul(
                        outf[:], sm[:, m, :], inv[:, m : m + 1]
                    )
                    nc.sync.dma_start(out_tbv[2 * m, :, :], outf[0:B, :])
                    nc.sync.dma_start(out_tbv[2 * m + 1, :, :], outf[B:128, :])

    nc.compile()
    return nc


def prep_inputs(features, captions, lenghts, emb_table, w_ih, w_hh,
                b_ih, b_hh, fc_w, fc_b):
    """host-side prep: dtype casts, weight transposes, gate reorder, shards"""
    f32 = np.float32
    features = np.ascontiguousarray(np.asarray(features), dtype=f32)
    captions = np.asarray(captions).astype(np.int32)
    emb_table = np.ascontiguousarray(np.asarray(emb_table), dtype=f32)
    w_ih = np.asarray(w_ih, dtype=f32)
    w_hh = np.asarray(w_hh, dtype=f32)
    bias = (np.asarray(b_ih, dtype=f32) + np.asarray(b_hh, dtype=f32))
    fc_w = np.asarray(fc_w, dtype=f32)
    fc_b = np.asarray(fc_b, dtype=f32)

    # gate reorder: torch order i,f,g,o -> our layout i,f,o,g
    perm = np.r_[0:512, 512:1024, 1536:2048, 1024:1536]
    w_ihT = np.ascontiguousarray(w_ih[perm].T).reshape(4, 128, G4)
    w_hhT = np.ascontiguousarray(w_hh[perm].T).reshape(4, 128, G4)
    bias = np.ascontiguousarray(np.broadcast_to(bias[perm][None, :], (128, G4)))

    # t-major row index table for the embedding gather (padded to 1408)
    idx = np.zeros(RXP, dtype=np.int32)
    idx[B : B * S] = captions.T.ravel()
    idx_all = np.ascontiguousarray(idx.reshape(NM_X, 128).T)  # [128, NM_X]

    iden = np.eye(128, dtype=f32)
    ones = np.ones((1, 128), dtype=f32)

    common = dict(
        features=features,
        emb_table=emb_table,
        idx_all=idx_all,
        w_ihT=w_ihT,
        w_hhT=w_hhT,
        bias=bias,
        iden=iden,
        iden_r=iden,
        ones=ones,
    )
    in_maps = []
    for c in range(NCORES):
        sl = slice(c * VS, (c + 1) * VS)
        m = dict(common)
        m["fc_wT"] = np.ascontiguousarray(fc_w[sl].T).reshape(4, 128, VS)
        m["fc_b"] = np.ascontiguousarray(fc_b[sl]).reshape(1, VS)
        in_maps.append(m)
    return in_maps


_NC_CACHE = {}


def _get_nc(n_cores=NCORES):
    if n_cores not in _NC_CACHE:
        _NC_CACHE[n_cores] = build_nc(n_cores)
    return _NC_CACHE[n_cores]


def kernel(features, captions, lenghts, emb_table, w_ih, w_hh,
           b_ih, b_hh, fc_w, fc_b, _trace=False, _results=None):
    in_maps = prep_inputs(features, captions, lenghts, emb_table,
                          w_ih, w_hh, b_ih, b_hh, fc_w, fc_b)
    nc = _get_nc(NCORES)
    res = run_bass_kernel_spmd(
        nc, in_maps, core_ids=list(range(NCORES)), trace=_trace
    )
    if _results is not None:
        _results.append(res)
    # assemble: each core's out is [B, T, VS]; concat along vocab
    full = np.concatenate([r["out"] for r in res.results], axis=2)
    return full
